# revision 18
# baseline (speedup 1.0000x reference)
"""Trainium2 Bass kernel for nn_LSH: ret[o] = sum_{s,a} x[s] * w[o,s,a].

x: [1, 4096] f32, weights: [512, 4096, 128] f32 -> ret: [512] f32.

PE-reduction variant: out_dim 512 is split 64-per-core across 8 cores and
weights are cast to bf16 on the host (2e-2 tolerance; halves HBM traffic).

Per core, weights are repacked so the TensorEngine does the whole
contraction: partitions = s within a 128-wide s-tile t (32 tiles), moving
columns = (o_local, a) for one 16-o quarter q.  A [128,1] stationary
x-tile makes matmul compute out[0, j] = sum_p x[t*128+p] * w[p, j];
accumulating over the 32 s-tiles in 4 PSUM banks ([1,512] fp32 each)
yields v[o_l*128+a] = sum_s x[s] w[o,s,a] for the quarter.  Quarters
alternate between two 4-bank sets so the DVE drain (segment-reduce over a,
PSUM->SBUF) of quarter q overlaps accumulation of q+1; each quarter's 16
outputs go to HBM via the idle SWDGE queue as soon as they are ready.

Stream layout: blocks of [128 s, 2048 (o_l,a)] bf16 (0.5 MiB), order
q-major then t; chunk DMAs are contiguous DRAM blocks (up to 6 MiB).
Each chunk is fetched as TWO column-half DMAs issued on the two HWDGE
queues (sync + scalar) concurrently, halving per-chunk latency and
keeping both rings on the same chunk.

Latency hiding: ~8 us of dummy warm-up matmuls release the PE HAM clock
gate (1.2 -> 2.4 GHz) during the initial DMA fill, and the final 3 MiB
of the stream is prefetched mid-stream into a dedicated SBUF tile so the
tail has no DMA round-trips -- once the body lands, PE/DVE finish from
SBUF in a few us.
"""

import sys

sys.path.insert(0, "/opt/trn_rl_repo")

import ml_dtypes
import numpy as np

import concourse.bass as bass
import concourse.mybir as mybir
import concourse.tile as tile
from concourse import bacc
from concourse.bass_utils import run_bass_kernel_spmd

BF16 = ml_dtypes.bfloat16

P = 128
O_PER_CORE = 64
N_CORES = 8
S = 4096
A = 128
NT = S // P  # 32 s-tiles
NQ = 4  # o quarters (16 o_locals each)
QCOLS = 16 * A  # 2048 moving cols per block
NBANK = 4  # PSUM banks per quarter (two alternating sets)
BANK = QCOLS // NBANK  # 512 cols per PSUM bank
NBLK = NQ * NT  # 128 stream blocks

# Body chunk schedule in blocks (1 block = 0.5 MiB bf16); head taper fills
# the pipeline progressively.  The final TAIL_BLOCKS stream into their own
# SBUF tile, prefetched mid-stream (no buffer-recycle dependency).
CHUNKS = [2, 4, 8] + [12] * 9
TAIL_BLOCKS = 6
assert sum(CHUNKS) + TAIL_BLOCKS == NBLK

# DRAM tensor per chunk-size class (in blocks): name -> (n_chunks, blocks)
SIZE_CLASSES = {
    "wa": (1, 2),
    "wb": (1, 4),
    "wc": (1, 8),
    "wd": (9, 12),
    "wt": (1, TAIL_BLOCKS),
}
CHUNK_SRC = [("wa", 0), ("wb", 0), ("wc", 0)] + [("wd", k) for k in range(9)]
# Prefetch the tail right after this body chunk's DMA is issued.
TAIL_PREFETCH_AFTER = 2

_CACHED_NC = None


def _build_nc():
    nc = bacc.Bacc(
        "TRN2",
        target_bir_lowering=False,
        debug=False,
        num_devices=N_CORES,
    )
    wts = {
        name: nc.dram_tensor(
            name, [n * P, blocks * QCOLS], mybir.dt.bfloat16, kind="ExternalInput"
        ).ap()
        for name, (n, blocks) in SIZE_CLASSES.items()
    }
    xs = nc.dram_tensor("xs", [P, NT], mybir.dt.bfloat16, kind="ExternalInput").ap()
    out = nc.dram_tensor("out", [1, O_PER_CORE], mybir.dt.float32,
                         kind="ExternalOutput").ap()

    with tile.TileContext(nc) as tc:
        with (
            tc.tile_pool(name="wp", bufs=3) as wp,
            tc.tile_pool(name="const", bufs=1) as constp,
            tc.tile_pool(name="psum", bufs=1, space="PSUM") as psp,
        ):
            xs_t = constp.tile([P, NT], mybir.dt.bfloat16)
            rh = constp.tile([1, O_PER_CORE], mybir.dt.float32)
            dummy = constp.tile([P, 256], mybir.dt.bfloat16)
            tail = constp.tile([P, TAIL_BLOCKS * QCOLS], mybir.dt.bfloat16)
            ps = [
                psp.tile([1, BANK], mybir.dt.float32, name=f"ps{b}")
                for b in range(2 * NBANK)
            ]

            # Constants go via SWDGE so the HWDGE queues carry only the
            # weight stream.
            nc.gpsimd.dma_start(xs_t[:], xs[:])

            # Warm-up: sustained dummy matmuls while the first chunks
            # stream in, so the HAM clock gate releases (1.2 -> 2.4 GHz)
            # before real work arrives.  Results land in the second bank
            # set, which quarter 1 resets via start=True.
            nc.vector.memset(dummy[:], 0.0)
            for wi in range(40):
                nc.tensor.matmul(
                    ps[NBANK + (wi % NBANK)][:, :128],
                    dummy[:, :1],
                    dummy[:, :128],
                    start=True,
                    stop=True,
                    skip_group_check=True,
                )

            def do_block(buf, b_local, blk):
                q, t = blk // NT, blk % NT
                bankset = (q % 2) * NBANK
                for j in range(NBANK):
                    nc.tensor.matmul(
                        ps[bankset + j][:],
                        xs_t[:, t : t + 1],
                        buf[:, b_local * QCOLS + j * BANK :
                            b_local * QCOLS + (j + 1) * BANK],
                        start=(t == 0),
                        stop=(t == NT - 1),
                        skip_group_check=True,
                    )
                if t == NT - 1:
                    # Quarter done: fold each bank over a into ret and
                    # store the quarter's 16 outputs via the idle SWDGE
                    # queue (keeps HWDGE rings free for the stream).
                    for j in range(NBANK):
                        seg = ps[bankset + j][:].rearrange("p (o a) -> p o a", a=A)
                        nc.vector.tensor_reduce(
                            rh[:, q * 16 + j * 4 : q * 16 + (j + 1) * 4],
                            seg,
                            axis=mybir.AxisListType.X,
                            op=mybir.AluOpType.add,
                        )
                    nc.gpsimd.dma_start(
                        out[:, q * 16 : (q + 1) * 16],
                        rh[:, q * 16 : (q + 1) * 16],
                    )

            blk = 0  # global block index (q = blk // NT, t = blk % NT)
            for ci, blocks in enumerate(CHUNKS):
                wt = wp.tile([P, max(CHUNKS) * QCOLS], mybir.dt.bfloat16, tag="wt")
                cols = blocks * QCOLS
                name, slot = CHUNK_SRC[ci]
                src = wts[name][slot * P : (slot + 1) * P, :]
                if ci >= len(CHUNKS) - 2:
                    # Last two chunks: four finer sub-DMAs so PE can start
                    # on the leading blocks ~7us before the chunk completes.
                    quarter = cols // 4
                    for sd in range(4):
                        e = nc.sync if sd % 2 == 0 else nc.scalar
                        e.dma_start(
                            wt[:, sd * quarter : (sd + 1) * quarter],
                            src[:, sd * quarter : (sd + 1) * quarter],
                        )
                else:
                    half = cols // 2
                    nc.sync.dma_start(wt[:, :half], src[:, :half])
                    nc.scalar.dma_start(wt[:, half:cols], src[:, half:])
                if ci == TAIL_PREFETCH_AFTER:
                    thalf = TAIL_BLOCKS * QCOLS // 2
                    nc.sync.dma_start(tail[:, :thalf], wts["wt"][:, :thalf])
                    nc.scalar.dma_start(tail[:, thalf:], wts["wt"][:, thalf:])
                for b_local in range(blocks):
                    do_block(wt, b_local, blk)
                    blk += 1
                if ci < len(CHUNKS):
                    # Keep the PE HAM activity window busy while waiting for
                    # the next chunk: accumulate +0 (zeros x zeros) into the
                    # currently-open bank.  Purely a clock-gate keep-alive;
                    # executes inside the would-be idle gap.  No chunk
                    # boundary coincides with a quarter end (blk % 32 != 0).
                    assert blk % NT != 0
                    live = ps[((blk // NT) % 2) * NBANK]
                    for _ in range(20):
                        nc.tensor.matmul(
                            live[:, :128],
                            dummy[:, :1],
                            dummy[:, :128],
                            start=False,
                            stop=False,
                            skip_group_check=True,
                        )
            for b_local in range(TAIL_BLOCKS):
                do_block(tail, b_local, blk)
                blk += 1
            assert blk == NBLK

    nc.compile()
    return nc


def _get_nc():
    global _CACHED_NC
    if _CACHED_NC is None:
        _CACHED_NC = _build_nc()
    return _CACHED_NC


def _in_maps(x, weights):
    x = np.ascontiguousarray(np.asarray(x, dtype=np.float32))
    weights = np.asarray(weights, dtype=np.float32)
    # xs[p, t] = x[t*128 + p]
    xs = np.ascontiguousarray(x.reshape(NT, P).T).astype(BF16)

    chunk_blocks = CHUNKS + [TAIL_BLOCKS]
    chunk_src = CHUNK_SRC + [("wt", 0)]
    maps = []
    for c in range(N_CORES):
        wc = weights[c * O_PER_CORE : (c + 1) * O_PER_CORE]  # [64, 4096, 128]
        # [q, o_l, t, p, a] -> [q, t, p, o_l, a] -> flat [128 blocks, 128, 2048]
        flat = (
            wc.reshape(NQ, 16, NT, P, A)
            .transpose(0, 2, 3, 1, 4)
            .reshape(NBLK, P, QCOLS)
            .astype(BF16)
        )
        m = {"xs": xs}
        arrs = {
            name: np.empty((n * P, blocks * QCOLS), dtype=BF16)
            for name, (n, blocks) in SIZE_CLASSES.items()
        }
        j = 0
        for ci, blocks in enumerate(chunk_blocks):
            name, slot = chunk_src[ci]
            arrs[name][slot * P : (slot + 1) * P, :] = (
                flat[j : j + blocks].transpose(1, 0, 2).reshape(P, blocks * QCOLS)
            )
            j += blocks
        assert j == NBLK
        m.update(arrs)
        maps.append(m)
    return maps


def run(x, weights, trace=False):
    """Run on hardware; returns (ret[512], BassKernelResults)."""
    nc = _get_nc()
    res = run_bass_kernel_spmd(
        nc, _in_maps(x, weights), list(range(N_CORES)), trace=trace
    )
    ret = np.concatenate(
        [res.results[c]["out"].reshape(O_PER_CORE) for c in range(N_CORES)]
    ).astype(np.float32)
    return ret, res


def kernel(x, weights):
    ret, _ = run(x, weights)
    return ret


# revision 19
# speedup vs baseline: 1.1529x; 1.1529x over previous
"""Trainium2 Bass kernel for nn_LSH: ret[o] = sum_{s,a} x[s] * w[o,s,a].

x: [1, 4096] f32, weights: [512, 4096, 128] f32 -> ret: [512] f32.

PE-reduction variant: out_dim 512 is split 64-per-core across 8 cores and
weights are cast to bf16 on the host (2e-2 tolerance; halves HBM traffic).

Per core, weights are repacked so the TensorEngine does the whole
contraction: partitions = s within a 128-wide s-tile t (32 tiles), moving
columns = (o_local, a) for one 16-o quarter q.  A [128,1] stationary
x-tile makes matmul compute out[0, j] = sum_p x[t*128+p] * w[p, j];
accumulating over the 32 s-tiles in 4 PSUM banks ([1,512] fp32 each)
yields v[o_l*128+a] = sum_s x[s] w[o,s,a] for the quarter.  Quarters
alternate between two 4-bank sets so the DVE drain (segment-reduce over a,
PSUM->SBUF) of quarter q overlaps accumulation of q+1; each quarter's 16
outputs go to HBM via the idle SWDGE queue as soon as they are ready.

Stream layout: blocks of [128 s, 2048 (o_l,a)] bf16 (0.5 MiB), order
q-major then t; chunk DMAs are contiguous DRAM blocks (up to 6 MiB).
Each chunk is fetched as TWO column-half DMAs issued on the two HWDGE
queues (sync + scalar) concurrently, halving per-chunk latency and
keeping both rings on the same chunk.

Latency hiding: ~8 us of dummy warm-up matmuls release the PE HAM clock
gate (1.2 -> 2.4 GHz) during the initial DMA fill, and the final 3 MiB
of the stream is prefetched mid-stream into a dedicated SBUF tile so the
tail has no DMA round-trips -- once the body lands, PE/DVE finish from
SBUF in a few us.
"""

import sys

sys.path.insert(0, "/opt/trn_rl_repo")

import ml_dtypes
import numpy as np

import concourse.bass as bass
import concourse.mybir as mybir
import concourse.tile as tile
from concourse import bacc
from concourse.bass_utils import run_bass_kernel_spmd

BF16 = ml_dtypes.bfloat16

P = 128
O_PER_CORE = 64
N_CORES = 8
S = 4096
A = 128
NT = S // P  # 32 s-tiles
NQ = 4  # o quarters (16 o_locals each)
QCOLS = 16 * A  # 2048 moving cols per block
NBANK = 4  # PSUM banks per quarter (two alternating sets)
BANK = QCOLS // NBANK  # 512 cols per PSUM bank
NBLK = NQ * NT  # 128 stream blocks

# Body chunk schedule in blocks (1 block = 0.5 MiB bf16); head taper fills
# the pipeline progressively.  The final TAIL_BLOCKS stream into their own
# SBUF tile, prefetched mid-stream (no buffer-recycle dependency).
CHUNKS = [2, 4, 8] + [12] * 9
TAIL_BLOCKS = 6
assert sum(CHUNKS) + TAIL_BLOCKS == NBLK

# DRAM tensor per chunk-size class (in blocks): name -> (n_chunks, blocks)
SIZE_CLASSES = {
    "wa": (1, 2),
    "wb": (1, 4),
    "wc": (1, 8),
    "wd": (9, 12),
    "wt": (1, TAIL_BLOCKS),
}
CHUNK_SRC = [("wa", 0), ("wb", 0), ("wc", 0)] + [("wd", k) for k in range(9)]
# Prefetch the tail right after this body chunk's DMA is issued.
TAIL_PREFETCH_AFTER = 2

_CACHED_NC = None


def _build_nc():
    nc = bacc.Bacc(
        "TRN2",
        target_bir_lowering=False,
        debug=False,
        num_devices=N_CORES,
    )
    wts = {
        name: nc.dram_tensor(
            name, [n * P, blocks * QCOLS], mybir.dt.bfloat16, kind="ExternalInput"
        ).ap()
        for name, (n, blocks) in SIZE_CLASSES.items()
    }
    xs = nc.dram_tensor("xs", [P, NT], mybir.dt.bfloat16, kind="ExternalInput").ap()
    out = nc.dram_tensor("out", [1, O_PER_CORE], mybir.dt.float32,
                         kind="ExternalOutput").ap()

    with tile.TileContext(nc) as tc:
        with (
            tc.tile_pool(name="wp", bufs=3) as wp,
            tc.tile_pool(name="const", bufs=1) as constp,
            tc.tile_pool(name="psum", bufs=1, space="PSUM") as psp,
        ):
            xs_t = constp.tile([P, NT], mybir.dt.bfloat16)
            rh = constp.tile([1, O_PER_CORE], mybir.dt.float32)
            dummy = constp.tile([P, 256], mybir.dt.bfloat16)
            tail = constp.tile([P, TAIL_BLOCKS * QCOLS], mybir.dt.bfloat16)
            ps = [
                psp.tile([1, BANK], mybir.dt.float32, name=f"ps{b}")
                for b in range(2 * NBANK)
            ]

            # Constants go via SWDGE so the HWDGE queues carry only the
            # weight stream.
            nc.gpsimd.dma_start(xs_t[:], xs[:])

            # Warm-up: sustained dummy matmuls while the first chunks
            # stream in, so the HAM clock gate releases (1.2 -> 2.4 GHz)
            # before real work arrives.  Results land in the second bank
            # set, which quarter 1 resets via start=True.
            nc.vector.memset(dummy[:], 0.0)
            for wi in range(72):
                nc.tensor.matmul(
                    ps[NBANK + (wi % NBANK)][:, :128],
                    dummy[:, :1],
                    dummy[:, :128],
                    start=True,
                    stop=True,
                    skip_group_check=True,
                )

            def do_block(buf, b_local, blk):
                q, t = blk // NT, blk % NT
                bankset = (q % 2) * NBANK
                for j in range(NBANK):
                    nc.tensor.matmul(
                        ps[bankset + j][:],
                        xs_t[:, t : t + 1],
                        buf[:, b_local * QCOLS + j * BANK :
                            b_local * QCOLS + (j + 1) * BANK],
                        start=(t == 0),
                        stop=(t == NT - 1),
                        skip_group_check=True,
                    )
                if t == NT - 1:
                    # Quarter done: fold each bank over a into ret and
                    # store the quarter's 16 outputs via the idle SWDGE
                    # queue (keeps HWDGE rings free for the stream).
                    for j in range(NBANK):
                        seg = ps[bankset + j][:].rearrange("p (o a) -> p o a", a=A)
                        nc.vector.tensor_reduce(
                            rh[:, q * 16 + j * 4 : q * 16 + (j + 1) * 4],
                            seg,
                            axis=mybir.AxisListType.X,
                            op=mybir.AluOpType.add,
                        )
                    nc.gpsimd.dma_start(
                        out[:, q * 16 : (q + 1) * 16],
                        rh[:, q * 16 : (q + 1) * 16],
                    )

            blk = 0  # global block index (q = blk // NT, t = blk % NT)
            for ci, blocks in enumerate(CHUNKS):
                wt = wp.tile([P, max(CHUNKS) * QCOLS], mybir.dt.bfloat16, tag="wt")
                cols = blocks * QCOLS
                name, slot = CHUNK_SRC[ci]
                src = wts[name][slot * P : (slot + 1) * P, :]
                half = cols // 2
                nc.sync.dma_start(wt[:, :half], src[:, :half])
                nc.scalar.dma_start(wt[:, half:cols], src[:, half:])
                if ci == TAIL_PREFETCH_AFTER:
                    thalf = TAIL_BLOCKS * QCOLS // 2
                    nc.sync.dma_start(tail[:, :thalf], wts["wt"][:, :thalf])
                    nc.scalar.dma_start(tail[:, thalf:], wts["wt"][:, thalf:])
                for b_local in range(blocks):
                    do_block(wt, b_local, blk)
                    blk += 1
                if 2 <= ci < len(CHUNKS) - 1:
                    # Keep the PE HAM activity window busy while waiting for
                    # the next chunk: accumulate +0 (zeros x zeros) into the
                    # currently-open bank.  Purely a clock-gate keep-alive;
                    # executes inside the would-be idle gap.  No chunk
                    # boundary coincides with a quarter end (blk % 32 != 0).
                    assert blk % NT != 0
                    live = ps[((blk // NT) % 2) * NBANK]
                    for _ in range(20):
                        nc.tensor.matmul(
                            live[:, :128],
                            dummy[:, :1],
                            dummy[:, :128],
                            start=False,
                            stop=False,
                            skip_group_check=True,
                        )
            for b_local in range(TAIL_BLOCKS):
                do_block(tail, b_local, blk)
                blk += 1
            assert blk == NBLK

    nc.compile()
    return nc


def _get_nc():
    global _CACHED_NC
    if _CACHED_NC is None:
        _CACHED_NC = _build_nc()
    return _CACHED_NC


def _in_maps(x, weights):
    x = np.ascontiguousarray(np.asarray(x, dtype=np.float32))
    weights = np.asarray(weights, dtype=np.float32)
    # xs[p, t] = x[t*128 + p]
    xs = np.ascontiguousarray(x.reshape(NT, P).T).astype(BF16)

    chunk_blocks = CHUNKS + [TAIL_BLOCKS]
    chunk_src = CHUNK_SRC + [("wt", 0)]
    maps = []
    for c in range(N_CORES):
        wc = weights[c * O_PER_CORE : (c + 1) * O_PER_CORE]  # [64, 4096, 128]
        # [q, o_l, t, p, a] -> [q, t, p, o_l, a] -> flat [128 blocks, 128, 2048]
        flat = (
            wc.reshape(NQ, 16, NT, P, A)
            .transpose(0, 2, 3, 1, 4)
            .reshape(NBLK, P, QCOLS)
            .astype(BF16)
        )
        m = {"xs": xs}
        arrs = {
            name: np.empty((n * P, blocks * QCOLS), dtype=BF16)
            for name, (n, blocks) in SIZE_CLASSES.items()
        }
        j = 0
        for ci, blocks in enumerate(chunk_blocks):
            name, slot = chunk_src[ci]
            arrs[name][slot * P : (slot + 1) * P, :] = (
                flat[j : j + blocks].transpose(1, 0, 2).reshape(P, blocks * QCOLS)
            )
            j += blocks
        assert j == NBLK
        m.update(arrs)
        maps.append(m)
    return maps


def run(x, weights, trace=False):
    """Run on hardware; returns (ret[512], BassKernelResults)."""
    nc = _get_nc()
    res = run_bass_kernel_spmd(
        nc, _in_maps(x, weights), list(range(N_CORES)), trace=trace
    )
    ret = np.concatenate(
        [res.results[c]["out"].reshape(O_PER_CORE) for c in range(N_CORES)]
    ).astype(np.float32)
    return ret, res


def kernel(x, weights):
    ret, _ = run(x, weights)
    return ret
